# revision 22
# baseline (speedup 1.0000x reference)
"""Distributed ImprovedDilatedAttention on 8 Trainium2 NeuronCores.

Problem: [2, 4096, 12, 64] q/k/v, 3 head groups with (segment, dilation) in
[(1024,1), (2048,2), (4096,4)]. Each (group, batch, segment, head) pair is an
independent dense 1024x1024 attention over head_dim 64 (m = g/r = 1024 for
every group): 56 problems total, 7 per core.

Host side packs one bf16 input block per problem, [128, 2568] = qT | kT | vp:
  qT [128, 1024] = (A' * Q)^T duplicated into both partition halves, where
      A' = 16*log2(e) pre-scales scores so S arrives as x = 128*log2(e^(s/8))
  kT [128, 1024] = K^T duplicated likewise (stationary operand for S^T)
  vp [128, 8, 65] = V' chunks, V' = [V | 1]; vp[j, c, :] = V'[c*128 + j];
      rows belonging to DVE-exp'd chunks are scaled by sqrt(2) (see below)
Device computes, per problem:
  S^T[kj, qi] = sum_d K^T[d,kj] Q'^T[d,qi]  (4 chunks of 2 kj-blocks; the two
      blocks of a chunk run concurrently on PE row strips 0-63 / 64-127)
  E: chunks alternate between two engines (both ~2us, running in parallel):
      ScalarE: E = exp(x * ln2/128)  (one [128, 2048] ACTIVATE)
      VectorE: EXP_BITS_ANT custom op -> bf16 bits of 2^((x-64)/128)
        = exp(s/8)/sqrt(2); the sqrt(2) is folded into those V' rows on host.
  out[qi, m] = sum_kj E[kj, qi] V'[kj, m]   (PV "quarters": 2 qi-blocks fully
      accumulated in PSUM, stationary = 128x128 E slices, moving = V')
PSUM: two single-buffered 4-bank pools; S chunks ping-pong between them so
the exp engines are never starved, and PV quarters borrow the just-freed
slot. PV of problem p is emitted interleaved with the S chunks of p+1.
out[:, 0:64] is the unnormalized O, col 64 is sumexp. Host divides and
scatters into the dilated positions (zeros elsewhere).
"""

import numpy as np

B, N, H, D = 2, 4096, 12, 64
SEG = [1024, 2048, 4096]
DIL = [1, 2, 4]
NGROUPS = 3
HPG = H // NGROUPS  # 4 heads per group
M = 1024            # dilated tokens per segment (g // r, same for all groups)
NPROB = 56
NCORES = 8
PPC = NPROB // NCORES  # 7 problems per core

# exp engine split: each kj-pair chunk is computed as two qi-half tiles;
# the lower half goes to ScalarE ("A"), the upper half to the VectorE
# EXP_BITS_ANT op ("D"). The DVE op's global 2^-0.5 factor is uniform in kj
# for a fixed qi column, so it cancels between softmax numerator and
# denominator -- no V' compensation needed.

APRIME = 16.0 * 1.4426950408889634  # 128 * 0.125 * log2(e)
ACT_SCALE = float(np.log(2.0) / 128.0)

# --- EXP_BITS_ANT custom DVE op -------------------------------------------
# bf16 bits of 2^((x - 64)/128), all-fp32 pipeline:
#   w = x + M2; k = w - M2            (= 128*rint(x/128), exact)
#   z = x - k                         (in [-64, 64])
#   bits = z*(z*c2 + c1) + k + c3;  out = int16(bits)  (round-to-nearest)
# true bits = k + 16000 + 128*2^((z+64)/128): single smooth branch, quadratic
# Remez fit ~0.32 bits. Total value err ~0.6% (bf16 floor is 0.39%).
EB_M2 = float(1.5 * 2**30)
EB_C2 = 0.002687508647645283
EB_C1 = 0.9950478872021531
EB_C3 = 16180.991964579245

_CACHE = {}


def _bf16():
    import ml_dtypes

    return ml_dtypes.bfloat16


def _exp_bits_reference(in0, in1, s0, s1, imm2):
    x = in0.astype(np.float32)
    w = (x + np.float32(s0)).astype(np.float32)
    k = (w - np.float32(s0)).astype(np.float32)
    z = (x - k).astype(np.float32)
    h2 = (z * ((z * np.float32(s1)).astype(np.float32) + np.float32(imm2))).astype(
        np.float32
    )
    c3 = np.asarray(in1, dtype=np.float32).reshape(in0.shape[0], -1)[:, :1]
    return ((h2 + k).astype(np.float32) + c3).astype(np.float32)


def _register_exp_bits():
    """Append EXP_BITS_ANT to concourse.dve_ops.OPS (idempotent)."""
    if "op" in _CACHE:
        return _CACHE["op"]
    from concourse import dve_ops
    from concourse.dve_spec import (
        Spec,
        Src0,
        C0,
        C1,
        C2,
        C3,
        _spill_c3_to_src1,
        lower,
        _has_src1,
    )
    from concourse.dve_uop import DveOpSpec

    name = "EXP_BITS_ANT"
    if any(o.name == name for o in dve_ops.OPS):
        op = next(o for o in dve_ops.OPS if o.name == name)
        _CACHE["op"] = op
        return op

    w = Src0 + C0
    k = w - C0
    z = Src0 - k
    h2 = z * (z * C1 + C2)
    spec = Spec(body=_spill_c3_to_src1((h2 + k) + C3), reference=_exp_bits_reference)

    row = dve_ops._CUSTOM_DVE_ROW_BASE + len(dve_ops.OPS)
    assert row < 0x20
    dve_ops._SUB_OPCODE_FOR_NAME[name] = row
    shas = {}
    for ver in ("v3", "v4"):
        try:
            s = DveOpSpec(
                name=name, opcode=row, uops=lower(spec, ver=ver), rd1_en=_has_src1(spec)
            )
            shas[ver] = s.sha(ver)
        except Exception:
            pass
    op = dve_ops.DveOp(name, spec, subdim=False, uops_sha=shas)
    dve_ops.OPS.append(op)
    dve_ops.CUSTOM_DVE_SPECS[name] = spec
    _CACHE["op"] = op
    return op


def _groups():
    for i, (g, r) in enumerate(zip(SEG, DIL)):
        yield i, g, r, i % r, N // g


def _pack(query, key, value):
    """-> packed input [56, 128, 2568] bf16 (qT*A' | kT | vp w/ sqrt2 comp)."""
    bf16 = _bf16()
    qs, ks, vs = [], [], []
    for i, g, r, off, s in _groups():
        idx = off + r * np.arange(g // r)
        hsl = slice(i * HPG, (i + 1) * HPG)

        def grab(x):
            return x.reshape(B, s, g, H, D)[:, :, idx][:, :, :, hsl, :]

        qg = grab(query) * np.float32(APRIME)  # pre-scale scores
        kg = grab(key)
        vg = grab(value)
        qT = np.ascontiguousarray(qg.transpose(0, 1, 3, 4, 2)).reshape(-1, D, M)
        kT = np.ascontiguousarray(kg.transpose(0, 1, 3, 4, 2)).reshape(-1, D, M)
        # duplicate into both partition halves for 2-way PE row tiling
        qs.append(np.concatenate([qT, qT], axis=1))  # [n, 128, M]
        ks.append(np.concatenate([kT, kT], axis=1))
        v65 = np.concatenate(
            [vg, np.ones((*vg.shape[:-1], 1), np.float32)], axis=-1
        )  # [B, s, m, hpg, 65]
        vp = np.ascontiguousarray(v65.transpose(0, 1, 3, 2, 4)).reshape(-1, M, 65)
        vp = np.ascontiguousarray(vp.reshape(-1, 8, 128, 65).transpose(0, 2, 1, 3))
        vs.append(vp)
    qTp = np.concatenate(qs).astype(bf16)   # [56, 128, 1024]
    kTp = np.concatenate(ks).astype(bf16)   # [56, 128, 1024]
    vpp = np.concatenate(vs).astype(bf16)   # [56, 128, 8, 65]
    return np.concatenate(
        [qTp, kTp, vpp.reshape(NPROB, 128, 520)], axis=2
    )  # [56, 128, 2568]


def _unpack(outT):
    """outT [56, 128, 8, 65] (j, c, m; qi = c*128 + j) -> full output."""
    o = outT.transpose(0, 2, 1, 3).reshape(NPROB, M, 65)  # [56, qi, 65]
    o = o[:, :, :64] / o[:, :, 64:65]  # [56, qi, 64]
    out = np.zeros((B, N, H, D), np.float32)
    ofs = 0
    for i, g, r, off, s in _groups():
        idx = off + r * np.arange(g // r)
        n_i = B * s * HPG
        og = o[ofs : ofs + n_i].reshape(B, s, HPG, M, D).transpose(0, 1, 3, 2, 4)
        out.reshape(B, s, g, H, D)[:, :, idx, i * HPG : (i + 1) * HPG, :] = og
        ofs += n_i
    return out


def _build(for_hw=True):
    import concourse.bacc as bacc
    import concourse.bass as bass
    import concourse.mybir as mybir
    import concourse.tile as tile

    eb_op = _register_exp_bits()

    f32 = mybir.dt.float32
    f16 = mybir.dt.float16
    bf = mybir.dt.bfloat16
    i16 = mybir.dt.int16
    nc = bacc.Bacc("TRN2", target_bir_lowering=False, debug=False,
                   enable_asserts=False)
    inx = nc.dram_tensor("inx", [PPC, 128, 2568], bf, kind="ExternalInput").ap()
    outT = nc.dram_tensor("outT", [PPC, 128, 8, 65], f32, kind="ExternalOutput").ap()

    with tile.TileContext(nc) as tc:
        with (
            tc.tile_pool(name="misc", bufs=1) as misc,
            tc.tile_pool(name="inp", bufs=4) as inp,
            tc.tile_pool(name="exps", bufs=4) as exps,
            tc.tile_pool(name="outp", bufs=4) as outp,
            tc.tile_pool(name="psum0", bufs=1, space=bass.MemorySpace.PSUM) as sP0,
            tc.tile_pool(name="psum1", bufs=1, space=bass.MemorySpace.PSUM) as sP1,
            tc.tile_pool(name="psum2", bufs=1, space=bass.MemorySpace.PSUM) as sP2,
            tc.tile_pool(name="psum3", bufs=1, space=bass.MemorySpace.PSUM) as sP3,
        ):
            pools = (sP0, sP1, sP2, sP3)
            # c3 scalar for the custom op + ACT exp-table warm-up (the ~2.7us
            # table load happens during the first input DMA, off the critical
            # path).
            c3t = misc.tile([128, 1], f32, tag="c3")
            nc.vector.memset(c3t, EB_C3)
            warm = misc.tile([128, 1], f32, tag="warm")
            nc.scalar.activation(
                warm, c3t, mybir.ActivationFunctionType.Exp, scale=0.0
            )

            def emit_s_half(qt, kt, eS, t, h, pool):
                # One qi-half of kj-pair t: blocks j0 = 2t (PE rows 0-63) and
                # j1 = 2t+1 (rows 64-127) run concurrently via row tiling.
                # A [128, 2, 512] f32 tile is 2 PSUM banks, so four pools
                # rotate: the S->exp->PV->copy chain of one slot overlaps
                # three other slots instead of one.
                j0, j1 = 2 * t, 2 * t + 1
                cs = slice(h * 512, (h + 1) * 512)
                sch = pool.tile([128, 2, 512], f32, tag="s")
                nc.tensor.matmul(
                    sch[:, 0, :],
                    kt[0:64, j0 * 128 : (j0 + 1) * 128],
                    qt[0:64, cs],
                    start=True, stop=True,
                    tile_position=(0, 0),
                )
                nc.tensor.matmul(
                    sch[:, 1, :],
                    kt[64:128, j1 * 128 : (j1 + 1) * 128],
                    qt[64:128, cs],
                    start=True, stop=True,
                    tile_position=(64, 0),
                )
                if h == 0:
                    nc.scalar.activation(
                        eS[:, j0 : j0 + 2, cs], sch,
                        mybir.ActivationFunctionType.Exp, scale=ACT_SCALE,
                    )
                else:
                    nc.vector._custom_dve(
                        eb_op,
                        out=eS[:, j0 : j0 + 2, cs].bitcast(i16),
                        in0=sch,
                        in1=c3t,
                        s0=EB_M2, s1=EB_C2, imm2=EB_C1,
                    )

            def make_half_pv(eSp, vptp, ot, half, ceng, last_dma=None):
                def emit(pool):
                    # qi blocks 4*half .. 4*half+3 fully accumulated over all
                    # 8 kj chunks; [128, 4, 128] f32 = one PSUM bank, each MM
                    # output block (128-float stride) stays inside it. One
                    # copy drains all four blocks.
                    pvt = pool.tile([128, 4, 128], f32, tag="s")
                    for b in range(4):
                        qb = 4 * half + b
                        for c in range(8):
                            nc.tensor.matmul(
                                pvt[:, b, 0:65],
                                eSp[:, c, qb * 128 : (qb + 1) * 128],
                                vptp[:, c, :],
                                start=(c == 0),
                                stop=(c == 7),
                            )
                    dst = ot[:, 4 * half : 4 * half + 4, :]
                    if ceng == "A":
                        nc.scalar.copy(out=dst, in_=pvt[:, :, 0:65])
                    else:
                        nc.vector.tensor_copy(out=dst, in_=pvt[:, :, 0:65])
                    if last_dma is not None:
                        nc.gpsimd.dma_start(out=last_dma, in_=ot)
                return emit

            from collections import deque

            pend = deque()
            for p in range(PPC):
                it = inp.tile([128, 2568], bf, tag="it")
                # first piece = Q + first-half K: S chunks t0/t1 depend only
                # on it, so the PE can start one DMA earlier
                nc.gpsimd.dma_start(out=it[:, 0:1536], in_=inx[p][:, 0:1536])
                nc.gpsimd.dma_start(out=it[:, 1536:2568], in_=inx[p][:, 1536:2568])
                qt = it[:, 0:1024]
                kt = it[:, 1024:2048]
                vpt = it[:, 2048:2568].rearrange("p (c m) -> p c m", m=65)

                eS = exps.tile([128, 8, M], bf, tag="eS")
                ot = outp.tile([128, 8, 65], f32, tag="ot")
                for s in range(8):  # slot = (kj-pair t, qi-half h)
                    t, h = s // 2, s % 2
                    pool = pools[s % 4]
                    emit_s_half(qt, kt, eS, t, h, pool)
                    # one PV half (4 qi-blocks) per four slots; alternate the
                    # hosting pools by problem parity to spread bank load
                    if s in ((1, 5) if p % 2 == 0 else (3, 7)) and pend:
                        pend.popleft()(pool)
                for i in range(2):
                    pend.append(
                        make_half_pv(
                            eS, vpt, ot, i, "A" if (p + i) % 2 == 0 else "D",
                            last_dma=outT[p] if i == 1 else None,
                        )
                    )

            i = 0
            while pend:
                pend.popleft()(pools[(2 * i + 1) % 4])
                i += 1

    nc.compile()
    if for_hw:
        from concourse.bass_interp import get_hw_module

        nc.m = get_hw_module(nc.m)
    return nc


def _numpy_fallback(query, key, value, causal):
    out = np.zeros((B, N, H, D), np.float32)
    for i, g, r, off, s in _groups():
        idx = off + r * np.arange(g // r)
        hsl = slice(i * HPG, (i + 1) * HPG)
        q = query.reshape(B, s, g, H, D)[:, :, idx][:, :, :, hsl, :]
        k = key.reshape(B, s, g, H, D)[:, :, idx][:, :, :, hsl, :]
        v = value.reshape(B, s, g, H, D)[:, :, idx][:, :, :, hsl, :]
        scores = np.einsum("bsqhd,bskhd->bshqk", q, k) / np.sqrt(D).astype(np.float32)
        if causal:
            mask = np.tril(np.ones((g // r, g // r), dtype=bool))
            scores = np.where(mask, scores, np.float32(np.finfo(np.float32).min))
        scores -= scores.max(axis=-1, keepdims=True)
        p = np.exp(scores)
        p /= p.sum(axis=-1, keepdims=True)
        o = np.einsum("bshqk,bskhd->bsqhd", p, v)
        out.reshape(B, s, g, H, D)[:, :, idx, hsl, :] = o
    return out


def _in_maps(query, key, value):
    inx = _pack(query, key, value)
    return [
        {"inx": np.ascontiguousarray(inx[k * PPC : (k + 1) * PPC])}
        for k in range(NCORES)
    ]


def kernel(query, key, value, is_causal):
    query = np.asarray(query, dtype=np.float32)
    key = np.asarray(key, dtype=np.float32)
    value = np.asarray(value, dtype=np.float32)
    causal = bool(np.asarray(is_causal).item()) if np.ndim(is_causal) == 0 else bool(
        is_causal
    )
    if causal:
        return _numpy_fallback(query, key, value, causal)

    from concourse import bass_utils

    if "nc" not in _CACHE:
        _CACHE["nc"] = _build()
    nc = _CACHE["nc"]

    res = bass_utils.run_bass_kernel_spmd(
        nc, _in_maps(query, key, value), core_ids=list(range(NCORES))
    )
    outT = np.concatenate([res.results[k]["outT"] for k in range(NCORES)])
    return _unpack(outT)


# revision 24
# speedup vs baseline: 1.0733x; 1.0733x over previous
"""Distributed ImprovedDilatedAttention on 8 Trainium2 NeuronCores.

Problem: [2, 4096, 12, 64] q/k/v, 3 head groups with (segment, dilation) in
[(1024,1), (2048,2), (4096,4)]. Each (group, batch, segment, head) pair is an
independent dense 1024x1024 attention over head_dim 64 (m = g/r = 1024 for
every group): 56 problems total, 7 per core.

Host side packs one bf16 input block per problem, [128, 2568] = qT | kT | vp:
  qT [128, 1024] = (A' * Q)^T duplicated into both partition halves, where
      A' = 16*log2(e) pre-scales scores so S arrives as x = 128*log2(e^(s/8))
  kT [128, 1024] = K^T duplicated likewise (stationary operand for S^T)
  vp [128, 8, 65] = V' chunks, V' = [V | 1]; vp[j, c, :] = V'[c*128 + j];
      rows belonging to DVE-exp'd chunks are scaled by sqrt(2) (see below)
Device computes, per problem:
  S^T[kj, qi] = sum_d K^T[d,kj] Q'^T[d,qi]  (4 chunks of 2 kj-blocks; the two
      blocks of a chunk run concurrently on PE row strips 0-63 / 64-127)
  E: chunks alternate between two engines (both ~2us, running in parallel):
      ScalarE: E = exp(x * ln2/128)  (one [128, 2048] ACTIVATE)
      VectorE: EXP_BITS_ANT custom op -> bf16 bits of 2^((x-64)/128)
        = exp(s/8)/sqrt(2); the sqrt(2) is folded into those V' rows on host.
  out[qi, m] = sum_kj E[kj, qi] V'[kj, m]   (PV "quarters": 2 qi-blocks fully
      accumulated in PSUM, stationary = 128x128 E slices, moving = V')
PSUM: two single-buffered 4-bank pools; S chunks ping-pong between them so
the exp engines are never starved, and PV quarters borrow the just-freed
slot. PV of problem p is emitted interleaved with the S chunks of p+1.
out[:, 0:64] is the unnormalized O, col 64 is sumexp. Host divides and
scatters into the dilated positions (zeros elsewhere).
"""

import numpy as np

B, N, H, D = 2, 4096, 12, 64
SEG = [1024, 2048, 4096]
DIL = [1, 2, 4]
NGROUPS = 3
HPG = H // NGROUPS  # 4 heads per group
M = 1024            # dilated tokens per segment (g // r, same for all groups)
NPROB = 56
NCORES = 8
PPC = NPROB // NCORES  # 7 problems per core

# exp engine split: each kj-pair chunk is computed as two qi-half tiles;
# the lower half goes to ScalarE ("A"), the upper half to the VectorE
# EXP_BITS_ANT op ("D"). The DVE op's global 2^-0.5 factor is uniform in kj
# for a fixed qi column, so it cancels between softmax numerator and
# denominator -- no V' compensation needed.

APRIME = 16.0 * 1.4426950408889634  # 128 * 0.125 * log2(e)
ACT_SCALE = float(np.log(2.0) / 128.0)

# --- EXP_BITS_ANT custom DVE op -------------------------------------------
# bf16 bits of 2^((x - 64)/128), all-fp32 pipeline:
#   w = x + M2; k = w - M2            (= 128*rint(x/128), exact)
#   z = x - k                         (in [-64, 64])
#   bits = z*(z*c2 + c1) + k + c3;  out = int16(bits)  (round-to-nearest)
# true bits = k + 16000 + 128*2^((z+64)/128): single smooth branch, quadratic
# Remez fit ~0.32 bits. Total value err ~0.6% (bf16 floor is 0.39%).
EB_M2 = float(1.5 * 2**30)
EB_C2 = 0.002687508647645283
EB_C1 = 0.9950478872021531
EB_C3 = 16180.991964579245

_CACHE = {}


def _bf16():
    import ml_dtypes

    return ml_dtypes.bfloat16


def _exp_bits_reference(in0, in1, s0, s1, imm2):
    x = in0.astype(np.float32)
    w = (x + np.float32(s0)).astype(np.float32)
    k = (w - np.float32(s0)).astype(np.float32)
    z = (x - k).astype(np.float32)
    h2 = (z * ((z * np.float32(s1)).astype(np.float32) + np.float32(imm2))).astype(
        np.float32
    )
    c3 = np.asarray(in1, dtype=np.float32).reshape(in0.shape[0], -1)[:, :1]
    return ((h2 + k).astype(np.float32) + c3).astype(np.float32)


def _register_exp_bits():
    """Append EXP_BITS_ANT to concourse.dve_ops.OPS (idempotent)."""
    if "op" in _CACHE:
        return _CACHE["op"]
    from concourse import dve_ops
    from concourse.dve_spec import (
        Spec,
        Src0,
        C0,
        C1,
        C2,
        C3,
        _spill_c3_to_src1,
        lower,
        _has_src1,
    )
    from concourse.dve_uop import DveOpSpec

    name = "EXP_BITS_ANT"
    if any(o.name == name for o in dve_ops.OPS):
        op = next(o for o in dve_ops.OPS if o.name == name)
        _CACHE["op"] = op
        return op

    w = Src0 + C0
    k = w - C0
    z = Src0 - k
    h2 = z * (z * C1 + C2)
    spec = Spec(body=_spill_c3_to_src1((h2 + k) + C3), reference=_exp_bits_reference)

    row = dve_ops._CUSTOM_DVE_ROW_BASE + len(dve_ops.OPS)
    assert row < 0x20
    dve_ops._SUB_OPCODE_FOR_NAME[name] = row
    shas = {}
    for ver in ("v3", "v4"):
        try:
            s = DveOpSpec(
                name=name, opcode=row, uops=lower(spec, ver=ver), rd1_en=_has_src1(spec)
            )
            shas[ver] = s.sha(ver)
        except Exception:
            pass
    op = dve_ops.DveOp(name, spec, subdim=False, uops_sha=shas)
    dve_ops.OPS.append(op)
    dve_ops.CUSTOM_DVE_SPECS[name] = spec
    _CACHE["op"] = op
    return op


def _groups():
    for i, (g, r) in enumerate(zip(SEG, DIL)):
        yield i, g, r, i % r, N // g


def _pack(query, key, value):
    """-> packed input [56, 128, 2568] bf16 (qT*A' | kT | vp w/ sqrt2 comp)."""
    bf16 = _bf16()
    qs, ks, vs = [], [], []
    for i, g, r, off, s in _groups():
        idx = off + r * np.arange(g // r)
        hsl = slice(i * HPG, (i + 1) * HPG)

        def grab(x):
            return x.reshape(B, s, g, H, D)[:, :, idx][:, :, :, hsl, :]

        qg = grab(query) * np.float32(APRIME)  # pre-scale scores
        kg = grab(key)
        vg = grab(value)
        qT = np.ascontiguousarray(qg.transpose(0, 1, 3, 4, 2)).reshape(-1, D, M)
        kT = np.ascontiguousarray(kg.transpose(0, 1, 3, 4, 2)).reshape(-1, D, M)
        # duplicate into both partition halves for 2-way PE row tiling
        qs.append(np.concatenate([qT, qT], axis=1))  # [n, 128, M]
        ks.append(np.concatenate([kT, kT], axis=1))
        v65 = np.concatenate(
            [vg, np.ones((*vg.shape[:-1], 1), np.float32)], axis=-1
        )  # [B, s, m, hpg, 65]
        vp = np.ascontiguousarray(v65.transpose(0, 1, 3, 2, 4)).reshape(-1, M, 65)
        vp = np.ascontiguousarray(vp.reshape(-1, 8, 128, 65).transpose(0, 2, 1, 3))
        vs.append(vp)
    qTp = np.concatenate(qs).astype(bf16)   # [56, 128, 1024]
    kTp = np.concatenate(ks).astype(bf16)   # [56, 128, 1024]
    vpp = np.concatenate(vs).astype(bf16)   # [56, 128, 8, 65]
    return np.concatenate(
        [qTp, kTp, vpp.reshape(NPROB, 128, 520)], axis=2
    )  # [56, 128, 2568]


def _unpack(outT):
    """outT [56, 128, 8, 65] (j, c, m; qi = c*128 + j) -> full output."""
    o = outT.transpose(0, 2, 1, 3).reshape(NPROB, M, 65)  # [56, qi, 65]
    o = o[:, :, :64] / o[:, :, 64:65]  # [56, qi, 64]
    out = np.zeros((B, N, H, D), np.float32)
    ofs = 0
    for i, g, r, off, s in _groups():
        idx = off + r * np.arange(g // r)
        n_i = B * s * HPG
        og = o[ofs : ofs + n_i].reshape(B, s, HPG, M, D).transpose(0, 1, 3, 2, 4)
        out.reshape(B, s, g, H, D)[:, :, idx, i * HPG : (i + 1) * HPG, :] = og
        ofs += n_i
    return out


def _build(for_hw=True):
    import concourse.bacc as bacc
    import concourse.bass as bass
    import concourse.mybir as mybir
    import concourse.tile as tile

    eb_op = _register_exp_bits()

    f32 = mybir.dt.float32
    f16 = mybir.dt.float16
    bf = mybir.dt.bfloat16
    i16 = mybir.dt.int16
    nc = bacc.Bacc("TRN2", target_bir_lowering=False, debug=False,
                   enable_asserts=False)
    inx = nc.dram_tensor("inx", [PPC, 128, 2568], bf, kind="ExternalInput").ap()
    outT = nc.dram_tensor("outT", [PPC, 128, 8, 65], f32, kind="ExternalOutput").ap()

    with tile.TileContext(nc) as tc:
        with (
            tc.tile_pool(name="misc", bufs=1) as misc,
            tc.tile_pool(name="inp", bufs=4) as inp,
            tc.tile_pool(name="exps", bufs=4) as exps,
            tc.tile_pool(name="outp", bufs=4) as outp,
            tc.tile_pool(name="psum0", bufs=1, space=bass.MemorySpace.PSUM) as sP0,
            tc.tile_pool(name="psum1", bufs=1, space=bass.MemorySpace.PSUM) as sP1,
            tc.tile_pool(name="psum2", bufs=1, space=bass.MemorySpace.PSUM) as sP2,
            tc.tile_pool(name="psum3", bufs=1, space=bass.MemorySpace.PSUM) as sP3,
        ):
            pools = (sP0, sP1, sP2, sP3)
            # c3 scalar for the custom op + ACT exp-table warm-up (the ~2.7us
            # table load happens during the first input DMA, off the critical
            # path).
            c3t = misc.tile([128, 1], f32, tag="c3")
            nc.vector.memset(c3t, EB_C3)
            warm = misc.tile([128, 1], f32, tag="warm")
            nc.scalar.activation(
                warm, c3t, mybir.ActivationFunctionType.Exp, scale=0.0
            )

            def emit_s_half(qt, kt, eS, t, h, pool):
                # One qi-half of kj-pair t: blocks j0 = 2t (PE rows 0-63) and
                # j1 = 2t+1 (rows 64-127) run concurrently via row tiling.
                # A [128, 2, 512] f32 tile is 2 PSUM banks, so four pools
                # rotate: the S->exp->PV->copy chain of one slot overlaps
                # three other slots instead of one.
                j0, j1 = 2 * t, 2 * t + 1
                cs = slice(h * 512, (h + 1) * 512)
                sch = pool.tile([128, 2, 512], f32, tag="s")
                nc.tensor.matmul(
                    sch[:, 0, :],
                    kt[0:64, j0 * 128 : (j0 + 1) * 128],
                    qt[0:64, cs],
                    start=True, stop=True,
                    tile_position=(0, 0),
                )
                nc.tensor.matmul(
                    sch[:, 1, :],
                    kt[64:128, j1 * 128 : (j1 + 1) * 128],
                    qt[64:128, cs],
                    start=True, stop=True,
                    tile_position=(64, 0),
                )
                if h == 0:
                    nc.scalar.activation(
                        eS[:, j0 : j0 + 2, cs], sch,
                        mybir.ActivationFunctionType.Exp, scale=ACT_SCALE,
                    )
                else:
                    nc.vector._custom_dve(
                        eb_op,
                        out=eS[:, j0 : j0 + 2, cs].bitcast(i16),
                        in0=sch,
                        in1=c3t,
                        s0=EB_M2, s1=EB_C2, imm2=EB_C1,
                    )

            def make_quarter(eSp, vptp, ot, qpair, last_dma=None):
                def emit(pool):
                    # qi blocks 2*qpair, 2*qpair+1 fully accumulated over
                    # all 8 kj chunks; block stride of 128 floats keeps each
                    # matmul output inside one PSUM bank
                    pvt = pool.tile([128, 2, 128], f32, tag="s")
                    for b in range(2):
                        qb = 2 * qpair + b
                        for c in range(8):
                            nc.tensor.matmul(
                                pvt[:, b, 0:65],
                                eSp[:, c, qb * 128 : (qb + 1) * 128],
                                vptp[:, c, :],
                                start=(c == 0),
                                stop=(c == 7),
                            )
                    dst = ot[:, 2 * qpair : 2 * qpair + 2, :]
                    if qpair % 2 == 0:
                        nc.scalar.copy(out=dst, in_=pvt[:, :, 0:65])
                    else:
                        nc.vector.tensor_copy(out=dst, in_=pvt[:, :, 0:65])
                    if last_dma is not None:
                        nc.sync.dma_start(out=last_dma, in_=ot)
                return emit

            from collections import deque

            pend = deque()
            for p in range(PPC):
                it = inp.tile([128, 2568], bf, tag="it")
                if p == 0:
                    # tiny first piece (Q lower half + K blocks 0-1) so the
                    # first S matmul starts as early as possible
                    nc.sync.dma_start(out=it[:, 0:512], in_=inx[p][:, 0:512])
                    nc.sync.dma_start(out=it[:, 1024:1280], in_=inx[p][:, 1024:1280])
                    nc.sync.dma_start(out=it[:, 512:1024], in_=inx[p][:, 512:1024])
                    nc.sync.dma_start(out=it[:, 1280:2568], in_=inx[p][:, 1280:2568])
                else:
                    # first piece = Q + first-half K: S chunks t0/t1 depend
                    # only on it, so the PE can start one DMA earlier
                    nc.sync.dma_start(out=it[:, 0:1536], in_=inx[p][:, 0:1536])
                    nc.sync.dma_start(out=it[:, 1536:2568], in_=inx[p][:, 1536:2568])
                qt = it[:, 0:1024]
                kt = it[:, 1024:2048]
                vpt = it[:, 2048:2568].rearrange("p (c m) -> p c m", m=65)

                eS = exps.tile([128, 8, M], bf, tag="eS")
                ot = outp.tile([128, 8, 65], f32, tag="ot")
                for s in range(8):  # slot = (kj-pair t, qi-half h)
                    t, h = s // 2, s % 2
                    pool = pools[s % 4]
                    emit_s_half(qt, kt, eS, t, h, pool)
                    # one PV quarter per two slots, spread over all 4 pools
                    if s in (1, 3, 4, 6) and pend:
                        pend.popleft()(pool)
                for i in range(4):
                    pend.append(
                        make_quarter(
                            eS, vpt, ot, i,
                            last_dma=outT[p] if i == 3 else None,
                        )
                    )

            i = 0
            while pend:
                pend.popleft()(pools[i % 4])
                i += 1

    nc.compile()
    if for_hw:
        from concourse.bass_interp import get_hw_module

        nc.m = get_hw_module(nc.m)
    return nc


def _numpy_fallback(query, key, value, causal):
    out = np.zeros((B, N, H, D), np.float32)
    for i, g, r, off, s in _groups():
        idx = off + r * np.arange(g // r)
        hsl = slice(i * HPG, (i + 1) * HPG)
        q = query.reshape(B, s, g, H, D)[:, :, idx][:, :, :, hsl, :]
        k = key.reshape(B, s, g, H, D)[:, :, idx][:, :, :, hsl, :]
        v = value.reshape(B, s, g, H, D)[:, :, idx][:, :, :, hsl, :]
        scores = np.einsum("bsqhd,bskhd->bshqk", q, k) / np.sqrt(D).astype(np.float32)
        if causal:
            mask = np.tril(np.ones((g // r, g // r), dtype=bool))
            scores = np.where(mask, scores, np.float32(np.finfo(np.float32).min))
        scores -= scores.max(axis=-1, keepdims=True)
        p = np.exp(scores)
        p /= p.sum(axis=-1, keepdims=True)
        o = np.einsum("bshqk,bskhd->bsqhd", p, v)
        out.reshape(B, s, g, H, D)[:, :, idx, hsl, :] = o
    return out


def _in_maps(query, key, value):
    inx = _pack(query, key, value)
    return [
        {"inx": np.ascontiguousarray(inx[k * PPC : (k + 1) * PPC])}
        for k in range(NCORES)
    ]


def kernel(query, key, value, is_causal):
    query = np.asarray(query, dtype=np.float32)
    key = np.asarray(key, dtype=np.float32)
    value = np.asarray(value, dtype=np.float32)
    causal = bool(np.asarray(is_causal).item()) if np.ndim(is_causal) == 0 else bool(
        is_causal
    )
    if causal:
        return _numpy_fallback(query, key, value, causal)

    from concourse import bass_utils

    if "nc" not in _CACHE:
        _CACHE["nc"] = _build()
    nc = _CACHE["nc"]

    res = bass_utils.run_bass_kernel_spmd(
        nc, _in_maps(query, key, value), core_ids=list(range(NCORES))
    )
    outT = np.concatenate([res.results[k]["outT"] for k in range(NCORES)])
    return _unpack(outT)
